# revision 1
# baseline (speedup 1.0000x reference)
"""GCN 2-layer message-passing encoder on 8 Trainium2 NeuronCores.

Math (matches reference):
    deg  = out-degree(src) + 1 (self loops);  dinv = deg^-1/2
    norm_e = dinv[src]*dinv[dst]   (factorized: prescale table rows by dinv,
                                    postscale aggregated rows by dinv)
    layer(x, w, b):  out[v] = dinv[v] * sum_{e->v} (dinv[src] * x[src] * w) + b
    out = layer2(relu(layer1(x)))

Strategy: shard destination nodes (and their incoming edges) across the 8
cores.  Per core, edges are sorted by dst and packed into 128-edge tiles that
stay within one 128-node "bucket"; a data-built one-hot matrix turns the
scatter-add into a PE matmul accumulating in PSUM.  Gathers of source rows use
indirect DMA (2048 rows per instruction).  Between layers the per-core node
shards are AllGathered so every core has the full table to gather from.
"""
import numpy as np

import concourse.bacc as bacc
import concourse.bass as bass
import concourse.mybir as mybir
import concourse.tile as tile
from concourse import library_config
from concourse.bass import IndirectOffsetOnAxis
from concourse.bass_utils import run_bass_kernel_spmd

P = 128
F32 = mybir.dt.float32
I32 = mybir.dt.int32
I16 = mybir.dt.int16

LAST_RESULTS = None  # test harness reads exec_time_ns from here


# ----------------------------------------------------------------- host side
CHUNK = 32768  # int16 index ceiling for dma_gather


def preprocess(edges, n_nodes, n_cores, group_tiles):
    """Sort edges by (dst bucket, src chunk) and pad into the per-core tiled
    schedule.  Every (bucket, chunk) run is a contiguous span of 128-edge
    tiles gathered by one dma_gather; schedule is identical across cores."""
    src = np.asarray(edges[:, 0]).astype(np.int64)
    dst = np.asarray(edges[:, 1]).astype(np.int64)
    N = n_nodes
    shard = N // n_cores
    nb = (shard + P - 1) // P
    nch = (N + CHUNK - 1) // CHUNK

    deg = np.bincount(src, minlength=N).astype(np.float32) + 1.0
    dinv = (deg ** -0.5).astype(np.float32)

    loop = np.arange(N, dtype=np.int64)
    all_src = np.concatenate([src, loop])
    all_dst = np.concatenate([dst, loop])
    etot = all_src.shape[0]

    core = all_dst // shard
    bucket = (all_dst % shard) // P
    chunk = all_src // CHUNK
    key = (core * nb + bucket) * nch + chunk
    order = np.argsort(key, kind="stable")
    s_src = all_src[order]
    s_key = key[order]
    slot = (all_dst[order] % shard) % P
    s_chunk = chunk[order]

    cnt = np.bincount(s_key, minlength=n_cores * nb * nch)
    cnt = cnt.reshape(n_cores, nb, nch)

    tbc = (cnt.max(axis=0) + P - 1) // P          # [nb, nch] tiles per run
    T = int(tbc.sum())
    run_t0 = np.concatenate([[0], np.cumsum(tbc.ravel())])[:-1].reshape(nb, nch)

    starts = np.concatenate([[0], np.cumsum(cnt.ravel())])[:-1].reshape(
        n_cores, nb, nch)
    s_core = s_key // (nb * nch)
    s_bucket = (s_key // nch) % nb
    pos = np.arange(etot) - starts[s_core, s_bucket, s_chunk]
    tile_of = run_t0[s_bucket, s_chunk] + pos // P

    slot_arr = np.full((n_cores, P, T), -1.0, np.float32)
    slot_arr[s_core, pos % P, tile_of] = slot.astype(np.float32)

    # dma_gather idx layout: within a run, edge i sits at partition i%16,
    # free column t0*8 + i//16 (relative to the run's tile base), value is
    # the chunk-relative row.  Replicated across the 8 Q7 stripes.
    idx16 = np.full((n_cores, 16, T * 8), -1, np.int16)
    idx16[s_core, pos % 16, run_t0[s_bucket, s_chunk] * 8 + pos // 16] = (
        s_src - s_chunk * CHUNK).astype(np.int16)
    # a run with zero edges on some core still needs >=1 valid index
    for c in range(n_cores):
        zb, zc = np.nonzero(cnt[c] == 0)
        idx16[c, 0, run_t0[zb, zc] * 8] = 0
    idx16 = np.tile(idx16, (1, 8, 1))             # [n_cores, 128, 8T]

    dinv_arr = np.zeros((n_cores, P, nb), np.float32)
    cc, bb, pp = np.meshgrid(np.arange(n_cores), np.arange(nb), np.arange(P),
                             indexing="ij")
    valid = (bb * P + pp) < shard
    g = cc * shard + bb * P + pp
    dinv_arr[cc[valid], pp[valid], bb[valid]] = dinv[g[valid]]

    runs = []                                     # (bucket, chunk, t0, ntl)
    runcnt = []
    for b in range(nb):
        for ch in range(nch):
            if tbc[b, ch] > 0:
                runs.append((b, ch, int(run_t0[b, ch]), int(tbc[b, ch])))
                runcnt.append(np.maximum(cnt[:, b, ch], 1))
    runcnt = np.stack(runcnt, axis=1).astype(np.int32)   # [n_cores, n_runs]
    bt0 = np.concatenate([[0], np.cumsum(tbc.sum(axis=1))])
    first = bt0[:-1]                              # first tile of bucket
    last = bt0[1:] - 1                            # last tile of bucket

    return dict(idx16=idx16, slot=slot_arr, dinv_grid=dinv_arr, dinv=dinv,
                T=T, shard=shard, nb=nb, nch=nch, runs=runs, runcnt=runcnt,
                ntl_max=int(tbc.max()), first=first, last=last)


# --------------------------------------------------------------- device side
def build_gcn(tc, sched, cfg):
    """Trace the full 2-layer GCN program into TileContext `tc`.

    cfg: dict(N, D, n_cores, group_tiles, use_w1, use_b1, use_w2, use_b2)
    Tensors are declared here with fixed names; see in_map construction.
    """
    from contextlib import ExitStack
    ctx = ExitStack()
    nc = tc.nc
    N, D = cfg["N"], cfg["D"]
    NC = cfg["n_cores"]
    GT = cfg["group_tiles"]
    T = sched["T"]
    shard, nb = sched["shard"], sched["nb"]
    last_pt = shard - (nb - 1) * P

    x_sh = nc.dram_tensor("x_shard", [shard, D], F32, kind="ExternalInput").ap()
    idx_t = nc.dram_tensor("idx", [P, 8 * T], I16, kind="ExternalInput").ap()
    # All f32 constants DVE reads are packed into one tensor loaded by one
    # DMA: TT-struct instructions have a single sync-wait slot, so every
    # DVE-read constant must arrive on one DMA-lane semaphore.
    meta_w = T + P + nb + 4 * D
    meta_t = nc.dram_tensor("meta", [P, meta_w], F32, kind="ExternalInput").ap()
    out_t = nc.dram_tensor("out", [shard, D], F32, kind="ExternalOutput").ap()
    n_runs = len(sched["runs"])
    rc_t = nc.dram_tensor("runcnt", [1, n_runs], I32, kind="ExternalInput").ap()

    dram = ctx.enter_context(tc.tile_pool(name="dram", bufs=1, space="DRAM"))
    xt_shd = dram.tile([shard, D], F32, name="xt_shd")
    xt_full = dram.tile([N, D], F32, addr_space="Shared", name="xt_full")
    h_shd = dram.tile([shard, D], F32, name="h_shd")
    h_full = dram.tile([N, D], F32, addr_space="Shared", name="h_full")

    const = ctx.enter_context(tc.tile_pool(name="const", bufs=1))
    idx_sb = const.tile([P, 8 * T], I16, name="idx_sb")
    meta_sb = const.tile([P, meta_w], F32, name="meta_sb")
    rc_sb = const.tile([1, n_runs], I32, name="rc_sb")
    nc.sync.dma_start(out=idx_sb[:], in_=idx_t[:])
    nc.sync.dma_start(out=meta_sb[:], in_=meta_t[:])
    nc.sync.dma_start(out=rc_sb[:], in_=rc_t[:])
    slot_sb = meta_sb[:, 0:T]
    iota_sb = meta_sb[:, T:T + P]
    dinv_sb = meta_sb[:, T + P:T + P + nb]
    wb_sb = {}
    for i, name in enumerate(("w1b", "b1b", "w2b", "b2b")):
        o = T + P + nb + i * D
        wb_sb[name] = meta_sb[:, o:o + D]

    groups = [list(range(NC))]
    nc.gpsimd.load_library(library_config.mlp)  # dma_gather lives in mlp lib

    # ---- phase 0: xt = dinv * (x * w1) on own shard, then AllGather
    ph = ctx.enter_context(tc.tile_pool(name="ph", bufs=4))
    for b in range(nb):
        pt = P if b < nb - 1 else last_pt
        xa = ph.tile([P, D], F32, tag="ph_x")
        nc.sync.dma_start(out=xa[:pt, :], in_=x_sh[b * P:b * P + pt, :])
        nc.vector.tensor_scalar(out=xa[:pt, :], in0=xa[:pt, :],
                                scalar1=dinv_sb[:pt, b:b + 1], scalar2=None,
                                op0=mybir.AluOpType.mult)
        if cfg["use_w1"]:
            nc.vector.tensor_tensor(out=xa[:pt, :], in0=xa[:pt, :],
                                    in1=wb_sb["w1b"][:pt, :],
                                    op=mybir.AluOpType.mult)
        nc.sync.dma_start(out=xt_shd[b * P:b * P + pt, :], in_=xa[:pt, :])
    nc.gpsimd.collective_compute(
        "AllGather", mybir.AluOpType.bypass, replica_groups=groups,
        ins=[xt_shd[:]], outs=[xt_full[:]])

    # ---- edge passes
    gp = ctx.enter_context(tc.tile_pool(name="gather", bufs=3))
    op = ctx.enter_context(tc.tile_pool(name="onehot", bufs=3))
    pp = ctx.enter_context(tc.tile_pool(name="psum", bufs=4, space="PSUM"))
    fp = ctx.enter_context(tc.tile_pool(name="flush", bufs=3))

    first, last = sched["first"], sched["last"]

    def flush(b, ps, layer):
        pt = P if b < nb - 1 else last_pt
        ft = fp.tile([P, D], F32, tag="flush")
        dv = dinv_sb[:pt, b:b + 1]
        nc.vector.tensor_scalar(out=ft[:pt, :], in0=ps[:pt, :], scalar1=dv,
                                scalar2=None, op0=mybir.AluOpType.mult)
        use_b = cfg["use_b1"] if layer == 1 else cfg["use_b2"]
        if use_b:
            bb = wb_sb["b1b" if layer == 1 else "b2b"]
            nc.vector.tensor_tensor(out=ft[:pt, :], in0=ft[:pt, :],
                                    in1=bb[:pt, :], op=mybir.AluOpType.add)
        if layer == 1:
            nc.vector.tensor_scalar(out=ft[:pt, :], in0=ft[:pt, :],
                                    scalar1=0.0, scalar2=dv,
                                    op0=mybir.AluOpType.max,
                                    op1=mybir.AluOpType.mult)
            if cfg["use_w2"]:
                nc.vector.tensor_tensor(out=ft[:pt, :], in0=ft[:pt, :],
                                        in1=wb_sb["w2b"][:pt, :],
                                        op=mybir.AluOpType.mult)
            nc.sync.dma_start(out=h_shd[b * P:b * P + pt, :], in_=ft[:pt, :])
        else:
            nc.sync.dma_start(out=out_t[b * P:b * P + pt, :], in_=ft[:pt, :])

    ntl_max = sched["ntl_max"]
    state = {"n_gt": 0}

    def edge_pass(table, layer):
        ps = None
        for r, (b, ch, t0, ntl) in enumerate(sched["runs"]):
            lo = ch * CHUNK
            hi = min(lo + CHUNK, N)
            gt = gp.tile([P, ntl_max, D], F32, tag="gt")
            # clear so rows skipped by the short gather (padding) hold zeros
            nc.vector.memset(gt[:], 0.0)
            rc = nc.gpsimd.alloc_register(f"rc_{layer}_{r}")
            nc.gpsimd.reg_load(rc, rc_sb[0:1, r:r + 1])
            nc.gpsimd.dma_gather(
                out_ap=gt[:, :ntl, :], in_ap=table[lo:hi, :],
                idxs_ap=idx_sb[:, t0 * 8:(t0 + ntl) * 8],
                num_idxs=ntl * P, num_idxs_reg=rc, elem_size=D)
            oh = op.tile([P, ntl, P], F32, tag="oh")
            nc.vector.tensor_tensor(
                out=oh[:],
                in0=iota_sb[:, None, :].broadcast_to([P, ntl, P]),
                in1=slot_sb[:, t0:t0 + ntl, None].broadcast_to([P, ntl, P]),
                op=mybir.AluOpType.is_equal)
            for j in range(ntl):
                t = t0 + j
                if t == first[b]:
                    ps = pp.tile([P, D], F32, tag="ps")
                nc.tensor.matmul(out=ps[:], lhsT=oh[:, j, :], rhs=gt[:, j, :],
                                 start=(t == first[b]), stop=(t == last[b]))
                if t == last[b]:
                    flush(b, ps, layer)

    edge_pass(xt_full, 1)
    nc.gpsimd.collective_compute(
        "AllGather", mybir.AluOpType.bypass, replica_groups=groups,
        ins=[h_shd[:]], outs=[h_full[:]])
    edge_pass(h_full, 2)
    ctx.close()


def pack_meta(sched, c, w1, b1, w2, b2):
    """[P, T + P + nb + 4D] f32: slot | iota | dinv | w1b | b1b | w2b | b2b."""
    T, nb = sched["T"], sched["nb"]
    D = w1.shape[0]
    iota = np.broadcast_to(np.arange(P, dtype=np.float32), (P, P))
    parts = [sched["slot"][c], iota, sched["dinv_grid"][c],
             np.broadcast_to(w1, (P, D)), np.broadcast_to(b1, (P, D)),
             np.broadcast_to(w2, (P, D)), np.broadcast_to(b2, (P, D))]
    return np.ascontiguousarray(np.concatenate(parts, axis=1, dtype=np.float32))


# ---------------------------------------------------------------- entry point
def _run(edges, x, weight1, bias1, weight2, bias2, n_cores=8, group_tiles=16,
         trace=False):
    global LAST_RESULTS
    x = np.ascontiguousarray(np.asarray(x, np.float32))
    N, D = x.shape
    sched = preprocess(np.asarray(edges), N, n_cores, group_tiles)
    shard = sched["shard"]

    w1 = np.asarray(weight1, np.float32).reshape(-1)
    b1 = np.asarray(bias1, np.float32).reshape(-1)
    w2 = np.asarray(weight2, np.float32).reshape(-1)
    b2 = np.asarray(bias2, np.float32).reshape(-1)
    cfg = dict(N=N, D=D, n_cores=n_cores, group_tiles=group_tiles,
               use_w1=not np.all(w1 == 1.0), use_b1=not np.all(b1 == 0.0),
               use_w2=not np.all(w2 == 1.0), use_b2=not np.all(b2 == 0.0))

    nc = bacc.Bacc("TRN2", target_bir_lowering=False, debug=False,
                   num_devices=n_cores)
    with tile.TileContext(nc) as tc:
        build_gcn(tc, sched, cfg)
    nc.compile()

    in_maps = []
    for c in range(n_cores):
        m = dict(
            x_shard=np.ascontiguousarray(x[c * shard:(c + 1) * shard]),
            idx=np.ascontiguousarray(sched["idx16"][c]),
            meta=pack_meta(sched, c, w1, b1, w2, b2),
            runcnt=np.ascontiguousarray(sched["runcnt"][c:c + 1]),
        )
        in_maps.append(m)

    LAST_RESULTS = run_bass_kernel_spmd(
        nc, in_maps, core_ids=list(range(n_cores)), trace=trace)
    out = np.concatenate([r["out"] for r in LAST_RESULTS.results], axis=0)
    return out


def kernel(edges, x, weight1, bias1, weight2, bias2):
    import os
    return _run(edges, x, weight1, bias1, weight2, bias2,
                trace=bool(os.environ.get("GCN_TRACE")))



# revision 8
# speedup vs baseline: 2.5931x; 2.5931x over previous
"""GCN 2-layer message-passing encoder on 8 Trainium2 NeuronCores.

Math (matches reference):
    deg  = out-degree(src) + 1 (self loops);  dinv = deg^-1/2
    norm_e = dinv[src]*dinv[dst]   (factorized: prescale table rows by dinv,
                                    postscale aggregated rows by dinv)
    layer(x, w, b):  out[v] = dinv[v] * sum_{e->v} (dinv[src] * x[src] * w) + b
    out = layer2(relu(layer1(x)))

Strategy: shard destination nodes (and their incoming edges) across the 8
cores.  Per core, edges are sorted by dst and packed into 128-edge tiles that
stay within one 128-node "bucket"; a data-built one-hot matrix turns the
scatter-add into a PE matmul accumulating in PSUM.  Gathers of source rows use
indirect DMA (2048 rows per instruction).  Between layers the per-core node
shards are AllGathered so every core has the full table to gather from.
"""
import numpy as np

import concourse.bacc as bacc
import concourse.bass as bass
import concourse.mybir as mybir
import concourse.tile as tile
from concourse import library_config
from concourse.bass import IndirectOffsetOnAxis
from concourse.bass_utils import run_bass_kernel_spmd

P = 128
F32 = mybir.dt.float32
BF16 = mybir.dt.bfloat16
I32 = mybir.dt.int32
I16 = mybir.dt.int16
NQ = 4  # SWDGE queues: gathers on different queues run on different Q7 pairs

LAST_RESULTS = None  # test harness reads exec_time_ns from here


# ----------------------------------------------------------------- host side
CHUNK = 32768  # int16 index ceiling for dma_gather


def preprocess(edges, n_nodes, n_cores, group_tiles):
    """Sort edges by (dst bucket, src chunk) and pad into the per-core tiled
    schedule.  Every (bucket, chunk) run is a contiguous span of 128-edge
    tiles gathered by one dma_gather; schedule is identical across cores."""
    src = np.asarray(edges[:, 0]).astype(np.int64)
    dst = np.asarray(edges[:, 1]).astype(np.int64)
    N = n_nodes
    shard = N // n_cores
    nb = (shard + P - 1) // P
    nch = (N + CHUNK - 1) // CHUNK

    deg = np.bincount(src, minlength=N).astype(np.float32) + 1.0
    dinv = (deg ** -0.5).astype(np.float32)

    loop = np.arange(N, dtype=np.int64)
    all_src = np.concatenate([src, loop])
    all_dst = np.concatenate([dst, loop])
    etot = all_src.shape[0]

    core = all_dst // shard
    bucket = (all_dst % shard) // P
    chunk = all_src // CHUNK
    key = (core * nb + bucket) * nch + chunk
    order = np.argsort(key, kind="stable")
    s_src = all_src[order]
    s_key = key[order]
    slot = (all_dst[order] % shard) % P
    s_chunk = chunk[order]

    cnt = np.bincount(s_key, minlength=n_cores * nb * nch)
    cnt = cnt.reshape(n_cores, nb, nch)

    tbc = (cnt.max(axis=0) + P - 1) // P          # [nb, nch] tiles per run
    T = int(tbc.sum())
    run_t0 = np.concatenate([[0], np.cumsum(tbc.ravel())])[:-1].reshape(nb, nch)

    starts = np.concatenate([[0], np.cumsum(cnt.ravel())])[:-1].reshape(
        n_cores, nb, nch)
    s_core = s_key // (nb * nch)
    s_bucket = (s_key // nch) % nb
    pos = np.arange(etot) - starts[s_core, s_bucket, s_chunk]
    tile_of = run_t0[s_bucket, s_chunk] + pos // P

    slot_arr = np.full((n_cores, P, T), -1.0, np.float32)
    slot_arr[s_core, pos % P, tile_of] = slot.astype(np.float32)

    # dma_gather idx layout: within a run, edge i sits at partition i%16,
    # free column t0*8 + i//16 (relative to the run's tile base), value is
    # the chunk-relative row.  Replicated across the 8 Q7 stripes.
    idx16 = np.full((n_cores, 16, T * 8), -1, np.int16)
    idx16[s_core, pos % 16, run_t0[s_bucket, s_chunk] * 8 + pos // 16] = (
        s_src - s_chunk * CHUNK).astype(np.int16)
    # a run with zero edges on some core still needs >=1 valid index
    for c in range(n_cores):
        zb, zc = np.nonzero(cnt[c] == 0)
        idx16[c, 0, run_t0[zb, zc] * 8] = 0
    idx16 = np.tile(idx16, (1, 8, 1))             # [n_cores, 128, 8T]

    dinv_arr = np.zeros((n_cores, P, nb), np.float32)
    cc, bb, pp = np.meshgrid(np.arange(n_cores), np.arange(nb), np.arange(P),
                             indexing="ij")
    valid = (bb * P + pp) < shard
    g = cc * shard + bb * P + pp
    dinv_arr[cc[valid], pp[valid], bb[valid]] = dinv[g[valid]]

    runs = []                                     # (bucket, chunk, t0, ntl)
    runcnt = []
    for b in range(nb):
        for ch in range(nch):
            if tbc[b, ch] > 0:
                runs.append((b, ch, int(run_t0[b, ch]), int(tbc[b, ch])))
                runcnt.append(np.maximum(cnt[:, b, ch], 1))
    runcnt = np.stack(runcnt, axis=1).astype(np.int32)   # [n_cores, n_runs]
    bt0 = np.concatenate([[0], np.cumsum(tbc.sum(axis=1))])
    first = bt0[:-1]                              # first tile of bucket
    last = bt0[1:] - 1                            # last tile of bucket

    return dict(idx16=idx16, slot=slot_arr, dinv_grid=dinv_arr, dinv=dinv,
                T=T, shard=shard, nb=nb, nch=nch, runs=runs, runcnt=runcnt,
                ntl_max=int(tbc.max()), first=first, last=last)


# --------------------------------------------------------------- device side
def build_gcn(tc, sched, cfg):
    """Trace the full 2-layer GCN program into TileContext `tc`.

    cfg: dict(N, D, n_cores, group_tiles, use_w1, use_b1, use_w2, use_b2)
    Tensors are declared here with fixed names; see in_map construction.
    """
    from contextlib import ExitStack
    ctx = ExitStack()
    nc = tc.nc
    N, D = cfg["N"], cfg["D"]
    NC = cfg["n_cores"]
    GT = cfg["group_tiles"]
    T = sched["T"]
    shard, nb = sched["shard"], sched["nb"]
    last_pt = shard - (nb - 1) * P

    x_sh = nc.dram_tensor("x_shard", [shard, D], F32, kind="ExternalInput").ap()
    idx_t = nc.dram_tensor("idx", [P, 8 * T], I16, kind="ExternalInput").ap()
    # All f32 constants DVE reads are packed into one tensor loaded by one
    # DMA: TT-struct instructions have a single sync-wait slot, so every
    # DVE-read constant must arrive on one DMA-lane semaphore.
    meta_w = T + P + nb + 4 * D
    meta_t = nc.dram_tensor("meta", [P, meta_w], F32, kind="ExternalInput").ap()
    out_t = nc.dram_tensor("out", [shard, D], F32, kind="ExternalOutput").ap()
    n_runs = len(sched["runs"])
    rc_t = nc.dram_tensor("runcnt", [1, n_runs], I32, kind="ExternalInput").ap()

    dram = ctx.enter_context(tc.tile_pool(name="dram", bufs=1, space="DRAM"))
    xt_shd = dram.tile([shard, D], BF16, name="xt_shd")
    xt_full = dram.tile([N, D], BF16, addr_space="Shared", name="xt_full")
    h_shd = dram.tile([shard, D], BF16, name="h_shd")
    h_full = dram.tile([N, D], BF16, addr_space="Shared", name="h_full")

    const = ctx.enter_context(tc.tile_pool(name="const", bufs=1))
    idx_sb = const.tile([P, 8 * T], I16, name="idx_sb")
    meta_sb = const.tile([P, meta_w], F32, name="meta_sb")
    rc_sb = const.tile([1, n_runs], I32, name="rc_sb")
    nc.sync.dma_start(out=idx_sb[:], in_=idx_t[:])
    nc.sync.dma_start(out=meta_sb[:], in_=meta_t[:])
    nc.sync.dma_start(out=rc_sb[:], in_=rc_t[:])
    slot_sb = meta_sb[:, 0:T]
    iota_sb = meta_sb[:, T:T + P]
    dinv_sb = meta_sb[:, T + P:T + P + nb]
    wb_sb = {}
    for i, name in enumerate(("w1b", "b1b", "w2b", "b2b")):
        o = T + P + nb + i * D
        wb_sb[name] = meta_sb[:, o:o + D]

    groups = [list(range(NC))]
    nc.gpsimd.load_library(library_config.mlp)  # dma_gather lives in mlp lib

    # ---- phase 0: xt = dinv * (x * w1) on own shard (bf16), then AllGather
    ph = ctx.enter_context(tc.tile_pool(name="ph", bufs=4))
    for b in range(nb):
        pt = P if b < nb - 1 else last_pt
        xa = ph.tile([P, D], F32, tag="ph_x")
        xb = ph.tile([P, D], BF16, tag="ph_xb")
        nc.sync.dma_start(out=xa[:pt, :], in_=x_sh[b * P:b * P + pt, :])
        if cfg["use_w1"]:
            nc.vector.tensor_scalar(out=xa[:pt, :], in0=xa[:pt, :],
                                    scalar1=dinv_sb[:pt, b:b + 1], scalar2=None,
                                    op0=mybir.AluOpType.mult)
            nc.vector.tensor_tensor(out=xb[:pt, :], in0=xa[:pt, :],
                                    in1=wb_sb["w1b"][:pt, :],
                                    op=mybir.AluOpType.mult)
        else:
            nc.vector.tensor_scalar(out=xb[:pt, :], in0=xa[:pt, :],
                                    scalar1=dinv_sb[:pt, b:b + 1], scalar2=None,
                                    op0=mybir.AluOpType.mult)
        nc.sync.dma_start(out=xt_shd[b * P:b * P + pt, :], in_=xb[:pt, :])
    nc.gpsimd.collective_compute(
        "AllGather", mybir.AluOpType.bypass, replica_groups=groups,
        ins=[xt_shd[:]], outs=[xt_full[:]])

    # ---- edge passes
    GBUFS = 6
    gp = ctx.enter_context(tc.tile_pool(name="gather", bufs=GBUFS))
    op = ctx.enter_context(tc.tile_pool(name="onehot", bufs=4))
    pp = ctx.enter_context(tc.tile_pool(name="psum", bufs=4, space="PSUM"))
    fp = ctx.enter_context(tc.tile_pool(name="flush", bufs=3))

    first, last = sched["first"], sched["last"]

    def flush(b, ps, layer):
        pt = P if b < nb - 1 else last_pt
        ft = fp.tile([P, D], F32, tag="flush")
        dv = dinv_sb[:pt, b:b + 1]
        nc.vector.tensor_scalar(out=ft[:pt, :], in0=ps[:pt, :], scalar1=dv,
                                scalar2=None, op0=mybir.AluOpType.mult)
        use_b = cfg["use_b1"] if layer == 1 else cfg["use_b2"]
        if use_b:
            bb = wb_sb["b1b" if layer == 1 else "b2b"]
            nc.vector.tensor_tensor(out=ft[:pt, :], in0=ft[:pt, :],
                                    in1=bb[:pt, :], op=mybir.AluOpType.add)
        if layer == 1:
            fb = fp.tile([P, D], BF16, tag="flush_b")
            if cfg["use_w2"]:
                nc.vector.tensor_scalar(out=ft[:pt, :], in0=ft[:pt, :],
                                        scalar1=0.0, scalar2=dv,
                                        op0=mybir.AluOpType.max,
                                        op1=mybir.AluOpType.mult)
                nc.vector.tensor_tensor(out=fb[:pt, :], in0=ft[:pt, :],
                                        in1=wb_sb["w2b"][:pt, :],
                                        op=mybir.AluOpType.mult)
            else:
                nc.vector.tensor_scalar(out=fb[:pt, :], in0=ft[:pt, :],
                                        scalar1=0.0, scalar2=dv,
                                        op0=mybir.AluOpType.max,
                                        op1=mybir.AluOpType.mult)
            nc.sync.dma_start(out=h_shd[b * P:b * P + pt, :], in_=fb[:pt, :])
        else:
            nc.sync.dma_start(out=out_t[b * P:b * P + pt, :], in_=ft[:pt, :])

    ntl_max = sched["ntl_max"]
    state = {"n_gt": 0}

    def edge_pass(table, layer):
        ps = None
        for r, (b, ch, t0, ntl) in enumerate(sched["runs"]):
            lo = ch * CHUNK
            hi = min(lo + CHUNK, N)
            gt = gp.tile([P, ntl_max, D], BF16, tag="gt")
            # stale rows beyond the gathered count are neutralized by the
            # zero one-hot columns; memset each pool buffer once so the
            # initial SBUF garbage can't be NaN/Inf (0*NaN = NaN).
            if state["n_gt"] < GBUFS:
                nc.vector.memset(gt[:], 0.0)
            state["n_gt"] += 1
            rc = nc.gpsimd.alloc_register(f"rc_{layer}_{r}")
            nc.gpsimd.reg_load(rc, rc_sb[0:1, r:r + 1])
            nc.gpsimd.dma_gather(
                out_ap=gt[:, :ntl, :], in_ap=table[lo:hi, :],
                idxs_ap=idx_sb[:, t0 * 8:(t0 + ntl) * 8],
                num_idxs=ntl * P, num_idxs_reg=rc, elem_size=D,
                queue_num=r % NQ)
            oh = op.tile([P, ntl, P], BF16, tag="oh")
            nc.vector.tensor_tensor(
                out=oh[:],
                in0=iota_sb[:, None, :].broadcast_to([P, ntl, P]),
                in1=slot_sb[:, t0:t0 + ntl, None].broadcast_to([P, ntl, P]),
                op=mybir.AluOpType.is_equal)
            for j in range(ntl):
                t = t0 + j
                if t == first[b]:
                    ps = pp.tile([P, D], F32, tag="ps")
                nc.tensor.matmul(out=ps[:], lhsT=oh[:, j, :], rhs=gt[:, j, :],
                                 start=(t == first[b]), stop=(t == last[b]))
                if t == last[b]:
                    flush(b, ps, layer)

    edge_pass(xt_full, 1)
    nc.gpsimd.collective_compute(
        "AllGather", mybir.AluOpType.bypass, replica_groups=groups,
        ins=[h_shd[:]], outs=[h_full[:]])
    edge_pass(h_full, 2)
    ctx.close()


def pack_meta(sched, c, w1, b1, w2, b2):
    """[P, T + P + nb + 4D] f32: slot | iota | dinv | w1b | b1b | w2b | b2b."""
    T, nb = sched["T"], sched["nb"]
    D = w1.shape[0]
    iota = np.broadcast_to(np.arange(P, dtype=np.float32), (P, P))
    parts = [sched["slot"][c], iota, sched["dinv_grid"][c],
             np.broadcast_to(w1, (P, D)), np.broadcast_to(b1, (P, D)),
             np.broadcast_to(w2, (P, D)), np.broadcast_to(b2, (P, D))]
    return np.ascontiguousarray(np.concatenate(parts, axis=1, dtype=np.float32))


# ---------------------------------------------------------------- entry point
def _run(edges, x, weight1, bias1, weight2, bias2, n_cores=8, group_tiles=16,
         trace=False):
    global LAST_RESULTS
    x = np.ascontiguousarray(np.asarray(x, np.float32))
    N, D = x.shape
    sched = preprocess(np.asarray(edges), N, n_cores, group_tiles)
    shard = sched["shard"]

    w1 = np.asarray(weight1, np.float32).reshape(-1)
    b1 = np.asarray(bias1, np.float32).reshape(-1)
    w2 = np.asarray(weight2, np.float32).reshape(-1)
    b2 = np.asarray(bias2, np.float32).reshape(-1)
    cfg = dict(N=N, D=D, n_cores=n_cores, group_tiles=group_tiles,
               use_w1=not np.all(w1 == 1.0), use_b1=not np.all(b1 == 0.0),
               use_w2=not np.all(w2 == 1.0), use_b2=not np.all(b2 == 0.0))

    nc = bacc.Bacc("TRN2", target_bir_lowering=False, debug=False,
                   num_devices=n_cores, num_swdge_queues=NQ)
    with tile.TileContext(nc) as tc:
        build_gcn(tc, sched, cfg)
    nc.compile()

    in_maps = []
    for c in range(n_cores):
        m = dict(
            x_shard=np.ascontiguousarray(x[c * shard:(c + 1) * shard]),
            idx=np.ascontiguousarray(sched["idx16"][c]),
            meta=pack_meta(sched, c, w1, b1, w2, b2),
            runcnt=np.ascontiguousarray(sched["runcnt"][c:c + 1]),
        )
        in_maps.append(m)

    LAST_RESULTS = run_bass_kernel_spmd(
        nc, in_maps, core_ids=list(range(n_cores)), trace=trace)
    out = np.concatenate([r["out"] for r in LAST_RESULTS.results], axis=0)
    return out


def kernel(edges, x, weight1, bias1, weight2, bias2):
    import os
    return _run(edges, x, weight1, bias1, weight2, bias2,
                trace=bool(os.environ.get("GCN_TRACE")))



# revision 15
# speedup vs baseline: 3.2549x; 1.2552x over previous
"""GCN 2-layer message-passing encoder on 8 Trainium2 NeuronCores.

Math (matches reference):
    deg  = out-degree(src) + 1 (self loops);  dinv = deg^-1/2
    norm_e = dinv[src]*dinv[dst]   (factorized: prescale table rows by dinv,
                                    postscale aggregated rows by dinv)
    layer(x, w, b):  out[v] = dinv[v] * sum_{e->v} (dinv[src] * x[src] * w) + b
    out = layer2(relu(layer1(x)))

Strategy: shard destination nodes (and their incoming edges) across the 8
cores.  Per core, edges (self-loops excluded) are sorted into 128-edge tiles
that stay within one 2-bucket "pair-cell" and one 32768-row src chunk; a
data-built one-hot (one per pair side, is_eq on DVE) turns the scatter-add
into PE matmuls accumulating in PSUM.  Source-row gathers run on the GpSimd
SWDGE gather ant; calls are merged per (8-bucket block x chunk) and issued
round-robin over 4 SWDGE queues so descriptor generation overlaps across Q7
core pairs.  Self-loops use a static identity matmul on the (prescaled) local
shard tile, direct-DMA'd.  Flushes run on the Scalar engine (Relu/Copy with
per-partition scale).  Tables are bf16; PSUM accumulates f32.  Between layers
the per-core node shards are AllGathered (split in two halves for overlap).
"""
import numpy as np

import concourse.bacc as bacc
import concourse.bass as bass
import concourse.mybir as mybir
import concourse.tile as tile
from concourse import library_config
from concourse.bass_utils import run_bass_kernel_spmd

P = 128
F32 = mybir.dt.float32
BF16 = mybir.dt.bfloat16
I16 = mybir.dt.int16
NQ = 4       # SWDGE queues (gathers on different queues use different Q7 pairs)
GB = 4       # buckets per block (gather-call merge granularity; PSUM live set)
CHUNK = 32768  # int16 index ceiling for dma_gather
GMAX = 7     # max tiles per dma_gather call

LAST_RESULTS = None  # test harness reads exec_time_ns from here


# ----------------------------------------------------------------- host side
def preprocess(edges, n_nodes, n_cores):
    """Build the static tiled schedule.

    Edges (no self-loops) are binned per (core, pair-of-buckets, src-chunk)
    cell; each cell is padded to whole 128-edge tiles (max over cores so the
    instruction stream is SPMD-identical).  Tiles are laid out block-major,
    then chunk, then pair, so one dma_gather covers a whole (block, chunk)
    run from a single table chunk.  Every tile gets two matmuls (one per pair
    side) with data-built one-hots; padding edges carry slot=-1 (zero
    one-hot column) and idx=0 (valid, harmless gather row).
    """
    src = np.asarray(edges[:, 0]).astype(np.int64)
    dst = np.asarray(edges[:, 1]).astype(np.int64)
    N = n_nodes
    C = n_cores
    shard = N // C
    nb = (shard + P - 1) // P
    npair = (nb + 1) // 2
    nblk = (nb + GB - 1) // GB
    nch = (N + CHUNK - 1) // CHUNK

    deg = np.bincount(src, minlength=N).astype(np.float32) + 1.0
    dinv = (deg ** -0.5).astype(np.float32)

    core = dst // shard
    bucket = (dst % shard) // P
    pair = bucket // 2
    chunk = src // CHUNK
    cell = (core * npair + pair) * nch + chunk
    order = np.argsort(cell, kind="stable")
    s_cell = cell[order]
    cnt = np.bincount(cell, minlength=C * npair * nch).reshape(C, npair, nch)

    tbc = (cnt.max(axis=0) + P - 1) // P            # [npair, nch] tiles/cell
    # tile order: block-major, chunk, then pair-within-block
    cell_t0 = np.zeros((npair, nch), np.int64)
    t = 0
    runs = []                                        # (B, ch, t0, ntl, lo, hi)
    for B in range(nblk):
        p0, p1 = B * GB // 2, min((B + 1) * GB // 2, npair)
        for ch in range(nch):
            r0 = t
            for pr in range(p0, p1):
                cell_t0[pr, ch] = t
                t += int(tbc[pr, ch])
            if t > r0:
                runs.append((B, ch, r0, t - r0,
                             ch * CHUNK, min((ch + 1) * CHUNK, N)))
    T = t

    # per-edge static position
    starts = np.concatenate([[0], np.cumsum(cnt.ravel())])[:-1].reshape(
        C, npair, nch)
    pos_in_cell = np.arange(src.shape[0]) - starts[
        core[order], pair[order], chunk[order]]
    # tile/slot per (sorted) edge
    e_tile = cell_t0[pair[order], chunk[order]] + pos_in_cell // P
    e_row = pos_in_cell % P
    e_core = core[order]
    e_side = (bucket[order] % 2).astype(np.int64)
    e_slot = ((dst[order] % shard) % P).astype(np.float32)
    e_idx = (src[order] - chunk[order] * CHUNK).astype(np.int16)

    # slot2 [C, P, T, 2] f32, padding -1
    slot2 = np.full((C, P, T, 2), -1.0, np.float32)
    slot2[e_core, e_row, e_tile, e_side] = e_slot

    # idx16 [C, 16, 8T] int16, padding 0.  Within a tile t, edge j (row
    # j = pos%128) sits at partition j%16, column t*8 + j//16.
    idx16 = np.zeros((C, 16, 8 * T), np.int16)
    idx16[e_core, e_row % 16, e_tile * 8 + e_row // 16] = e_idx
    idx16 = np.tile(idx16, (1, 8, 1))                # [C, 128, 8T]

    # dinv grids [C, P, nb] (and squared) for flush scales
    dinv_arr = np.zeros((C, P, nb), np.float32)
    cc, bb, pp = np.meshgrid(np.arange(C), np.arange(nb), np.arange(P),
                             indexing="ij")
    valid = (bb * P + pp) < shard
    g = cc * shard + bb * P + pp
    dinv_arr[cc[valid], pp[valid], bb[valid]] = dinv[g[valid]]

    # per-bucket last tile (for matmul stop flags): last tile of its pair
    pair_last = np.full(npair, -1, np.int64)
    for pr in range(npair):
        tl = [cell_t0[pr, ch] + tbc[pr, ch] - 1
              for ch in range(nch) if tbc[pr, ch] > 0]
        if tl:
            pair_last[pr] = max(tl)

    # tile -> (pair) map for matmul emission
    tile_pair = np.zeros(T, np.int64)
    for pr in range(npair):
        for ch in range(nch):
            t0 = cell_t0[pr, ch]
            tile_pair[t0:t0 + tbc[pr, ch]] = pr

    return dict(idx16=idx16, slot2=slot2, dinv=dinv_arr, T=T, shard=shard,
                nb=nb, npair=npair, nblk=nblk, nch=nch, runs=runs,
                pair_last=pair_last, tile_pair=tile_pair,
                ntl_max=int(max((r[3] for r in runs), default=0)))


# --------------------------------------------------------------- device side
def build_gcn(tc, sched, cfg):
    from contextlib import ExitStack
    ctx = ExitStack()
    nc = tc.nc
    N, D = cfg["N"], cfg["D"]
    NC = cfg["n_cores"]
    T = sched["T"]
    shard, nb, nblk = sched["shard"], sched["nb"], sched["nblk"]
    last_pt = shard - (nb - 1) * P
    ntl_max = sched["ntl_max"]
    runs = sched["runs"]
    pair_last = sched["pair_last"]

    x_sh = nc.dram_tensor("x_shard", [shard, D], F32, kind="ExternalInput").ap()
    idx_t = nc.dram_tensor("idx", [P, 8 * T], I16, kind="ExternalInput").ap()
    # all DVE/ACT-read constants in one tensor -> one DMA-lane semaphore
    meta_w = 2 * T + P + 1 + 2 * nb + 4 * D
    meta_t = nc.dram_tensor("meta", [P, meta_w], F32, kind="ExternalInput").ap()
    out_t = nc.dram_tensor("out", [shard, D], F32, kind="ExternalOutput").ap()

    dram = ctx.enter_context(tc.tile_pool(name="dram", bufs=1, space="DRAM"))
    xt_shd = dram.tile([shard, D], BF16, name="xt_shd")
    xt_full = dram.tile([N, D], BF16, addr_space="Shared", name="xt_full")
    h_shd = dram.tile([shard, D], BF16, name="h_shd")
    h_full = dram.tile([N, D], BF16, addr_space="Shared", name="h_full")

    const = ctx.enter_context(tc.tile_pool(name="const", bufs=1))
    idx_sb = const.tile([P, 8 * T], I16, name="idx_sb")
    meta_sb = const.tile([P, meta_w], F32, name="meta_sb")
    ident = const.tile([P, P], BF16, name="ident")
    nc.sync.dma_start(out=idx_sb[:], in_=idx_t[:])
    nc.sync.dma_start(out=meta_sb[:], in_=meta_t[:])
    slot2_sb = meta_sb[:, 0:2 * T]
    iota_sb = meta_sb[:, 2 * T:2 * T + P]
    colio_sb = meta_sb[:, 2 * T + P:2 * T + P + 1]
    dinv_sb = meta_sb[:, 2 * T + P + 1:2 * T + P + 1 + nb]
    dinv2_sb = meta_sb[:, 2 * T + P + 1 + nb:2 * T + P + 1 + 2 * nb]
    wb_sb = {}
    for i, name in enumerate(("w1b", "b1b", "w2b", "b2b")):
        o = 2 * T + P + 1 + 2 * nb + i * D
        wb_sb[name] = meta_sb[:, o:o + D]

    groups = [list(range(NC))]
    nc.gpsimd.load_library(library_config.mlp)  # dma_gather lives in mlp lib

    # static identity (self-loop scatter): I[p,q] = (q == p)
    nc.vector.tensor_tensor(
        out=ident[:], in0=iota_sb[:, :],
        in1=colio_sb[:, 0:1].broadcast_to([P, P]),
        op=mybir.AluOpType.is_equal)

    # ---- phase 0: xt = dinv * (x * w1) on own shard (bf16), AllGather halves
    ph = ctx.enter_context(tc.tile_pool(name="ph", bufs=4))
    for b in range(nb):
        pt = P if b < nb - 1 else last_pt
        xa = ph.tile([P, D], F32, tag="ph_x")
        xb = ph.tile([P, D], BF16, tag="ph_xb")
        nc.sync.dma_start(out=xa[:pt, :], in_=x_sh[b * P:b * P + pt, :])
        nc.vector.tensor_scalar(out=xb[:pt, :], in0=xa[:pt, :],
                                scalar1=dinv_sb[:pt, b:b + 1], scalar2=None,
                                op0=mybir.AluOpType.mult)
        if cfg["use_w1"]:
            nc.vector.tensor_tensor(out=xb[:pt, :], in0=xb[:pt, :],
                                    in1=wb_sb["w1b"][:pt, :],
                                    op=mybir.AluOpType.mult)
        nc.sync.dma_start(out=xt_shd[b * P:b * P + pt, :], in_=xb[:pt, :])
    nc.gpsimd.collective_compute(
        "AllGather", mybir.AluOpType.bypass, replica_groups=groups,
        ins=[xt_shd[:]], outs=[xt_full[:]])

    # ---- edge passes
    gp = ctx.enter_context(tc.tile_pool(name="gather", bufs=5))
    op = ctx.enter_context(tc.tile_pool(name="onehot", bufs=3))
    pp = ctx.enter_context(tc.tile_pool(name="psum", bufs=8, space="PSUM"))
    fp = ctx.enter_context(tc.tile_pool(name="flush", bufs=4))
    sp = ctx.enter_context(tc.tile_pool(name="selfx", bufs=4))

    state = {"q": 0}
    runs_by_block = {}
    for (B, ch, t0, ntl, lo, hi) in runs:
        runs_by_block.setdefault(B, []).append((ch, t0, ntl, lo, hi))

    def flush(b, ps, layer):
        pt = P if b < nb - 1 else last_pt
        dv = dinv_sb[:pt, b:b + 1]
        if layer == 1:
            ft = fp.tile([P, D], F32, tag="ft1")
            fb = fp.tile([P, D], BF16, tag="fb")
            nc.vector.tensor_scalar(out=ft[:pt, :], in0=ps[:pt, :],
                                    scalar1=dv, scalar2=None,
                                    op0=mybir.AluOpType.mult)
            if cfg["use_b1"]:
                nc.vector.tensor_tensor(out=ft[:pt, :], in0=ft[:pt, :],
                                        in1=wb_sb["b1b"][:pt, :],
                                        op=mybir.AluOpType.add)
            nc.vector.tensor_scalar(out=fb[:pt, :], in0=ft[:pt, :],
                                    scalar1=0.0, scalar2=dv,
                                    op0=mybir.AluOpType.max,
                                    op1=mybir.AluOpType.mult)
            if cfg["use_w2"]:
                nc.vector.tensor_tensor(out=fb[:pt, :], in0=fb[:pt, :],
                                        in1=wb_sb["w2b"][:pt, :],
                                        op=mybir.AluOpType.mult)
            nc.sync.dma_start(out=h_shd[b * P:b * P + pt, :], in_=fb[:pt, :])
        else:
            ft = fp.tile([P, D], F32, tag="ft")
            nc.vector.tensor_scalar(out=ft[:pt, :], in0=ps[:pt, :],
                                    scalar1=dv, scalar2=None,
                                    op0=mybir.AluOpType.mult)
            if cfg["use_b2"]:
                nc.vector.tensor_tensor(out=ft[:pt, :], in0=ft[:pt, :],
                                        in1=wb_sb["b2b"][:pt, :],
                                        op=mybir.AluOpType.add)
            nc.sync.dma_start(out=out_t[b * P:b * P + pt, :], in_=ft[:pt, :])

    def edge_pass(table_full, table_shd, layer):
        tile_pair = sched["tile_pair"]
        for B in range(nblk):
            b0, b1 = B * GB, min((B + 1) * GB, nb)
            ps = {}
            # self-loop: ps[b] = I^T @ xt_rows (prescaled rows; start=True)
            for b in range(b0, b1):
                pt = P if b < nb - 1 else last_pt
                xt_t = sp.tile([P, D], BF16, tag="sx")
                nc.sync.dma_start(out=xt_t[:pt, :],
                                  in_=table_shd[b * P:b * P + pt, :])
                ps[b] = pp.tile([P, D], F32, tag="ps", name=f"ps{b % (2 * GB)}")
                nc.tensor.matmul(out=ps[b][:], lhsT=ident[:pt, :],
                                 rhs=xt_t[:pt, :], start=True,
                                 stop=(pair_last[b // 2] < 0))
            for (ch, t0, ntl, lo, hi) in runs_by_block.get(B, []):
                gt = gp.tile([P, ntl_max, D], BF16, tag="gt")
                for k0 in range(0, ntl, GMAX):
                    kn = min(GMAX, ntl - k0)
                    nc.gpsimd.dma_gather(
                        out_ap=gt[:, k0:k0 + kn, :], in_ap=table_full[lo:hi, :],
                        idxs_ap=idx_sb[:, (t0 + k0) * 8:(t0 + k0 + kn) * 8],
                        num_idxs=kn * P, num_idxs_reg=kn * P, elem_size=D,
                        queue_num=state["q"] % NQ)
                    state["q"] += 1
                oh = op.tile([P, ntl_max, 2, P], BF16, tag="oh")
                sl = slot2_sb[:, t0 * 2:(t0 + ntl) * 2].rearrange(
                    "p (t s) -> p t s", s=2)
                nc.vector.tensor_tensor(
                    out=oh[:, :ntl, :, :],
                    in0=iota_sb[:, None, None, :].broadcast_to([P, ntl, 2, P]),
                    in1=sl[:, :, :, None].broadcast_to([P, ntl, 2, P]),
                    op=mybir.AluOpType.is_equal)
                for j in range(ntl):
                    t = t0 + j
                    pr = int(tile_pair[t])
                    for s in range(2):
                        bkt = pr * 2 + s
                        if bkt >= nb:
                            continue
                        nc.tensor.matmul(
                            out=ps[bkt][:], lhsT=oh[:, j, s, :],
                            rhs=gt[:, j, :], start=False,
                            stop=(t == pair_last[pr]))
            for b in range(b0, b1):
                flush(b, ps[b], layer)
        if layer == 1:
            nc.gpsimd.collective_compute(
                "AllGather", mybir.AluOpType.bypass, replica_groups=groups,
                ins=[h_shd[:]], outs=[h_full[:]])

    edge_pass(xt_full, xt_shd, 1)
    edge_pass(h_full, h_shd, 2)
    ctx.close()


def pack_meta(sched, c, w1, b1, w2, b2):
    """[P, 2T+P+1+2nb+4D] f32: slot2 | iota | colio | dinv | dinv2 | w/b."""
    T, nb = sched["T"], sched["nb"]
    D = w1.shape[0]
    iota = np.broadcast_to(np.arange(P, dtype=np.float32), (P, P))
    colio = np.arange(P, dtype=np.float32)[:, None]
    dv = sched["dinv"][c]
    parts = [sched["slot2"][c].reshape(P, 2 * T), iota, colio, dv, dv * dv,
             np.broadcast_to(w1, (P, D)), np.broadcast_to(b1, (P, D)),
             np.broadcast_to(w2, (P, D)), np.broadcast_to(b2, (P, D))]
    return np.ascontiguousarray(np.concatenate(parts, axis=1, dtype=np.float32))


# ---------------------------------------------------------------- entry point
def _run(edges, x, weight1, bias1, weight2, bias2, n_cores=8, trace=False):
    global LAST_RESULTS
    x = np.ascontiguousarray(np.asarray(x, np.float32))
    N, D = x.shape
    sched = preprocess(np.asarray(edges), N, n_cores)
    shard = sched["shard"]

    w1 = np.asarray(weight1, np.float32).reshape(-1)
    b1 = np.asarray(bias1, np.float32).reshape(-1)
    w2 = np.asarray(weight2, np.float32).reshape(-1)
    b2 = np.asarray(bias2, np.float32).reshape(-1)
    cfg = dict(N=N, D=D, n_cores=n_cores,
               use_w1=not np.all(w1 == 1.0), use_b1=not np.all(b1 == 0.0),
               use_w2=not np.all(w2 == 1.0), use_b2=not np.all(b2 == 0.0))

    nc = bacc.Bacc("TRN2", target_bir_lowering=False, debug=False,
                   num_devices=n_cores, num_swdge_queues=NQ)
    with tile.TileContext(nc) as tc:
        build_gcn(tc, sched, cfg)
    nc.compile()

    in_maps = []
    for c in range(n_cores):
        m = dict(
            x_shard=np.ascontiguousarray(x[c * shard:(c + 1) * shard]),
            idx=np.ascontiguousarray(sched["idx16"][c]),
            meta=pack_meta(sched, c, w1, b1, w2, b2),
        )
        in_maps.append(m)

    LAST_RESULTS = run_bass_kernel_spmd(
        nc, in_maps, core_ids=list(range(n_cores)), trace=trace)
    out = np.concatenate([r["out"] for r in LAST_RESULTS.results], axis=0)
    return out


def kernel(edges, x, weight1, bias1, weight2, bias2):
    import os
    return _run(edges, x, weight1, bias1, weight2, bias2,
                trace=bool(os.environ.get("GCN_TRACE")))
